# revision 31
# baseline (speedup 1.0000x reference)
"""Self-contained Trainium2 Bass kernel for the GQA attention module.

Sharding: tensor-parallel over heads. Core c owns q-heads [4c..4c+4) and
kv-head c, computes its partial of (attn @ wo); the host sums the 8
partials (the "all-reduce after wo" done host-side during unshard).

Device design (see build_nc):
  - x arrives pre-transposed (xT [DIM, B*S]); Q/K/V projections use one
    packed weight [wq/8 | wv | wk] so K and V share a full 128-row
    M-tile (V^T lands on partitions 0:64 untouched, K^T on 64:128 gets
    RoPE; Q is RoPE'd via cos/sin tables + a pair-swap permutation
    matmul). Projection blocks are emitted just-in-time between
    attention chunks so their DMA/compute overlaps the pipeline.
  - scores are computed transposed (S^T [k, q]) so softmax's P^T is
    directly the moving operand of the PV matmul, and the softmax
    denominator comes free via a ones-column appended to V. The score
    pair for the two heads of a pair runs K=64 row-tiled on PE tiles
    T0/T8 concurrently (kTz_lo/kTz_hi hold K^T in the matching
    partition halves, zero-padded so full-K use also works).
  - mask tiles are classified host-side: all-(-inf) tiles are skipped,
    all-zero tiles skip the mask multiply, and mixed (diagonal) tiles
    are column-trimmed: scores/exp/PV run only on live columns and the
    mask multiply touches only the genuinely mixed sub-block.
  - softmax denominators for all 4 (pair, half) combos of a chunk are
    DMA-gathered to partitions 0/32/64/96 of one tile and Ln/Exp'd in
    two batched ACT ops; the reciprocal rows are broadcast back through
    K=128 selector matmuls.
  - each chunk's tail (denominators, at-normalization, wo matmuls +
    output stores) is deferred and drained into the next chunk's kt
    loop with a ~3-iteration delay, hiding the denominator latency
    chain and filling exp-gated PE slack. PSUM fits in 8 banks via two
    shared ring pools ([128,2,512]x2 and [65,512]x4).
"""
import sys
import types

sys.path.insert(0, "/opt/trn_rl_repo")

import numpy as np
import ml_dtypes


def _install_axon_hook_shim():
    import antenv

    if "antenv.axon_hooks" in sys.modules:
        return
    m = types.ModuleType("antenv.axon_hooks")
    m._hook = None

    def set_axon_ntff_profile_hook(h):
        m._hook = h

    def get_axon_ntff_profile_hook():
        return m._hook

    m.set_axon_ntff_profile_hook = set_axon_ntff_profile_hook
    m.get_axon_ntff_profile_hook = get_axon_ntff_profile_hook
    sys.modules["antenv.axon_hooks"] = m
    antenv.axon_hooks = m
    try:
        from trn_agent_boot.trn_boot import _ntff_profile_via_ctypes

        hook = _ntff_profile_via_ctypes("/opt/axon/libaxon_pjrt.so")
        if hook is not None:
            m.set_axon_ntff_profile_hook(hook)
    except Exception:
        pass


_install_axon_hook_shim()

import concourse.bass as bass
import concourse.mybir as mybir
import concourse.tile as tile
from concourse.bass_utils import run_bass_kernel_spmd

BF16 = mybir.dt.bfloat16
F32 = mybir.dt.float32

B, S, DIM = 2, 2048, 2048
N_HEADS, N_KV_HEADS, HEAD_DIM = 32, 8, 64
N_CORES = 8
HPC = N_HEADS // N_CORES  # 4 q heads per core
BS = B * S  # 4096 rows
NKT = S // 128  # 16 k tiles per batch
NQC = S // 512  # 4 q chunks per batch
NNT = BS // 512  # 8 projection column blocks
NEG_THRESH = -1e4

EXP = mybir.ActivationFunctionType.Exp
LN = mybir.ActivationFunctionType.Ln


def _patched_drain_and_barrier(self, tick_clock, wait_clock):
    # walrus (CoreV3) only accepts one sync-wait on the tile exit drain;
    # split the accumulated waits across single-wait nops.
    nc = self.nc
    drain_inst = nc.sync.drain()
    wait_clock.add_sem_waits(
        drain_inst.ins, tile.ScopedClock({None: tick_clock.global_clock})
    )
    si = drain_inst.ins.sync_info
    sw = list(si.on_wait) if si and si.on_wait else []
    if len(sw) > 1:
        si.on_wait = [sw[0]]
        for w in sw[1:]:
            n2 = nc.sync.nop(nofuse=True)
            if n2.ins.sync_info is None:
                n2.ins.sync_info = mybir.SyncInfo(on_wait=[w], on_update=[])
            else:
                n2.ins.sync_info.on_wait = [w]
    nc.all_engine_barrier()
    assert self.sems is not None
    popped = nc._tile_sem_poison_stack.pop()
    assert popped is self._sem_poison
    nc.clear_and_free_semaphores(list(self.sems.allocated().values()))
    nc.all_engine_barrier()


tile.TileContext._drain_and_barrier = _patched_drain_and_barrier


def _split_multi_waits(nc):
    """walrus (this build) accepts at most one sync-wait per instruction;
    move extra waits onto same-engine nops inserted just before."""
    n_split = 0
    for f in nc.m.functions:
        for blk in f.blocks:
            new_insts = []
            for inst in blk.instructions:
                si = getattr(inst, "sync_info", None)
                if si is not None and si.on_wait and len(si.on_wait) > 1:
                    extra = list(si.on_wait[:-1])
                    si.on_wait = [si.on_wait[-1]]
                    for w in extra:
                        nop = mybir.InstNoOp(
                            name=nc.get_next_instruction_name(), ins=[], outs=[]
                        )
                        nop.engine = inst.engine
                        nop.sync_info = mybir.SyncInfo(on_wait=[w], on_update=[])
                        new_insts.append(nop)
                        n_split += 1
                new_insts.append(inst)
            blk.instructions[:] = new_insts
    return n_split


def build_nc(classes, debug_phase=None):
    """classes[kt][qc]: ('z',) all-live / ('n',) dead / ('m', c0, c1) with
    q-columns [0,c0) dead, [c0,c1) mixed (mask applied), [c1,512) live."""
    nc = bass.Bass("TRN2", target_bir_lowering=False, debug=False, num_devices=N_CORES)

    xT_d = nc.dram_tensor("xT", [DIM, BS], BF16, kind="ExternalInput")
    # packed [wq*(1/8) | wv | wk]: V at output partitions 0:64, K at 64:128
    wqkv_d = nc.dram_tensor("wqkv_c", [DIM, 384], BF16, kind="ExternalInput")
    wo_d = nc.dram_tensor("wo_c", [HPC * HEAD_DIM, DIM], BF16, kind="ExternalInput")
    maskT_d = nc.dram_tensor("maskT", [S, S], BF16, kind="ExternalInput")
    cosd_d = nc.dram_tensor("cosd", [128, BS], BF16, kind="ExternalInput")
    sind_d = nc.dram_tensor("sind", [128, BS], BF16, kind="ExternalInput")
    perm_d = nc.dram_tensor("perm", [128, 128], BF16, kind="ExternalInput")
    eye64_d = nc.dram_tensor("eye64", [64, 64], BF16, kind="ExternalInput")
    out_d = nc.dram_tensor("out_c", [BS, DIM], BF16, kind="ExternalOutput")

    # chunk schedule: C_i = (qc, b); chunk (qc, b) needs proj block
    # nt = b*4 + qc (its own Q rows) and all earlier nts of batch b (K/V).
    chunk_order = [(qc, b) for qc in range(NQC) for b in range(B)]
    chunk_order.sort(key=lambda cb: (cb[0], cb[1]))
    # processing order (0,0),(0,1),(1,0),... -> nt need order 0,4,1,5,2,6,3,7
    nt_for_chunk = [b * NQC + qc for (qc, b) in chunk_order]

    with tile.TileContext(nc) as tc:
        with (
            tc.tile_pool(name="persist", bufs=1) as persist,
            tc.tile_pool(name="stream", bufs=3) as stream,
            tc.tile_pool(name="small", bufs=5) as small,
            tc.tile_pool(name="maskpool", bufs=16) as mp,
            tc.tile_pool(name="ps_big", bufs=2, space="PSUM") as ps_big,
            tc.tile_pool(name="ps_sm", bufs=4, space="PSUM") as ps_sm,
        ):
            # ---- persistent tensors ----
            wqkv_sb = persist.tile([128, NKT, 384], BF16, tag="wqkv")
            wo_sb = persist.tile([128, 2, DIM], BF16, tag="wo")
            perm_sb = persist.tile([128, 128], BF16, tag="perm")
            eye64_sb = persist.tile([64, 64], BF16, tag="eye64")
            q_sb = persist.tile([128, 2, NNT * 512], BF16, tag="q")  # Q^T
            # K^T zero-padded to K=128 so scores run in full (non-tiled)
            # PE mode: kTz_lo = [K^T; 0] for even heads, kTz_hi = [0; K^T].
            kTz_lo = persist.tile([128, BS], BF16, tag="kTlo")
            kTz_hi = persist.tile([128, BS], BF16, tag="kThi")
            v_sb = persist.tile([128, B * NKT, 65], BF16, tag="v")  # [V|1]
            at_sb = persist.tile([128, 2, BS], BF16, tag="at")  # A^T
            # selector rows for the K=128 denominator-broadcast matmuls
            sel_sb = persist.tile([128, 4, 64], BF16, tag="sel")
            dbuf = persist.tile([128, 512], F32, tag="dbuf")  # d rows at 0/32/64/96
            lgb = persist.tile([128, 512], F32, tag="lgb")
            recb = persist.tile([128, 512], BF16, tag="recb")

            for tch in range(4):
                eng = nc.gpsimd if tch % 2 == 0 else nc.sync
                eng.dma_start(
                    wqkv_sb[:, 4 * tch : 4 * tch + 4, :],
                    wqkv_d[4 * tch * 128 : (4 * tch + 4) * 128, :].rearrange(
                        "(t p) m -> p t m", p=128
                    ),
                )
            nc.gpsimd.dma_start(perm_sb[:], perm_d[:])
            nc.gpsimd.dma_start(eye64_sb[:], eye64_d[:])
            nc.gpsimd.memset(v_sb[:, :, 64:65], 1.0)
            nc.gpsimd.memset(kTz_lo[64:128, :], 0.0)
            nc.gpsimd.memset(kTz_hi[0:64, :], 0.0)
            nc.gpsimd.memset(sel_sb[:], 0.0)
            for i in range(4):
                nc.gpsimd.memset(sel_sb[32 * i : 32 * i + 1, i, :], 1.0)
            nc.gpsimd.memset(dbuf[:], 1.0)
            # prefetch all mixed-mask tiles
            all_mks = {}
            for qc in range(NQC):
                for kt in range(NKT):
                    cl = classes[kt][qc]
                    if cl[0] == "m":
                        c0, c1 = cl[1], cl[2]
                        mk = mp.tile(
                            [128, c1 - c0], BF16, tag=f"mk{c1 - c0}", name="mk"
                        )
                        nc.gpsimd.dma_start(
                            mk[:],
                            maskT_d[
                                kt * 128 : (kt + 1) * 128,
                                qc * 512 + c0 : qc * 512 + c1,
                            ],
                        )
                        all_mks[(kt, qc)] = mk
            # wo weights aren't needed until the first deferred wo drain
            nc.gpsimd.dma_start(wo_sb[:], wo_d.rearrange("(t p) m -> p t m", p=128))

            # ---- projection block emission (interleaved between chunks) ----
            xblks = {}

            def load_nt(nt, spread=False):
                cs = slice(nt * 512, (nt + 1) * 512)
                xblk = stream.tile([128, NKT, 512], BF16, tag="xblk")
                for tch in range(4):
                    eng = nc.scalar if (spread and tch % 2) else nc.sync
                    eng.dma_start(
                        xblk[:, 4 * tch : 4 * tch + 4, :],
                        xT_d[4 * tch * 128 : (4 * tch + 4) * 128, cs].rearrange(
                            "(t p) n -> p t n", p=128
                        ),
                    )
                cosb = stream.tile([128, 512], BF16, tag="cosb")
                sinb = stream.tile([128, 512], BF16, tag="sinb")
                nc.sync.dma_start(cosb[:], cosd_d[:, cs])
                nc.sync.dma_start(sinb[:], sind_d[:, cs])
                xblks[nt] = (xblk, cosb, sinb)

            def emit_proj(nt):
                cs = slice(nt * 512, (nt + 1) * 512)
                xblk, cosb, sinb = xblks.pop(nt)
                # Q: 2 M-tiles (the 1/8 scale is folded into wq host-side)
                for mt in range(2):
                    psqb = ps_big.tile([128, 2, 512], F32, tag="big", name="psq")
                    psq = psqb[:, 0, :]
                    for kt in range(NKT):
                        nc.tensor.matmul(
                            psq,
                            wqkv_sb[:, kt, mt * 128 : (mt + 1) * 128],
                            xblk[:, kt, :],
                            start=(kt == 0),
                            stop=(kt == NKT - 1),
                        )
                    q_tmp = small.tile([128, 512], BF16, tag="q_tmp")
                    nc.scalar.copy(q_tmp[:], psq)
                    psw = ps_sm.tile([128, 512], F32, tag="sm", name="psw")
                    nc.tensor.matmul(psw[:], perm_sb[:], q_tmp[:])
                    v1 = small.tile([128, 512], BF16, tag="v1")
                    nc.vector.tensor_mul(v1[:], q_tmp[:], cosb[:])
                    v2 = small.tile([128, 512], BF16, tag="v2")
                    nc.vector.tensor_mul(v2[:], psw[:], sinb[:])
                    nc.vector.tensor_add(q_sb[:, mt, cs], v1[:], v2[:])
                # fused [V | K] projection: V^T on partitions 0:64 (no RoPE,
                # transposed as-is), K^T on 64:128 (RoPE'd)
                pskb = ps_big.tile([128, 2, 512], F32, tag="big", name="psk")
                psk = pskb[:, 0, :]
                for kt in range(NKT):
                    nc.tensor.matmul(
                        psk,
                        wqkv_sb[:, kt, 256:384],
                        xblk[:, kt, :],
                        start=(kt == 0),
                        stop=(kt == NKT - 1),
                    )
                k_tmp = small.tile([128, 512], BF16, tag="k_tmp")
                nc.scalar.copy(k_tmp[:], psk)
                pskw = ps_sm.tile([128, 512], F32, tag="sm", name="pskw")
                nc.tensor.matmul(
                    pskw[64:128, :],
                    perm_sb[64:128, 64:128],
                    k_tmp[64:128, :],
                )
                kv1 = small.tile([128, 512], BF16, tag="kv1")
                nc.vector.tensor_mul(
                    kv1[64:128, :], k_tmp[64:128, :], cosb[64:128, :]
                )
                kv2 = small.tile([128, 512], BF16, tag="kv2")
                nc.vector.tensor_mul(
                    kv2[64:128, :], pskw[64:128, :], sinb[64:128, :]
                )
                nc.vector.tensor_add(
                    kTz_hi[64:128, cs], kv1[64:128, :], kv2[64:128, :]
                )
                nc.gpsimd.dma_start(kTz_lo[0:64, cs], kTz_hi[64:128, cs])
                for jt in range(4):
                    pst = ps_sm.tile([128, 512], BF16, tag="sm", name="pst")
                    nc.tensor.transpose(
                        pst[:, 0:64], k_tmp[0:64, jt * 128 : (jt + 1) * 128],
                        eye64_sb[:],
                    )
                    rc = nt * 4 + jt
                    nc.scalar.copy(v_sb[:, rc, 0:64], pst[:, 0:64])

            # ---- main pipeline over chunks ----
            deferred = []

            def drain(n):
                for _ in range(n):
                    if deferred:
                        deferred.pop(0)()

            load_nt(nt_for_chunk[0], spread=True)
            load_nt(nt_for_chunk[1], spread=True)
            for ci in range(len(chunk_order)):
                if ci + 2 < len(chunk_order):
                    load_nt(nt_for_chunk[ci + 2])
                emit_proj(nt_for_chunk[ci])

            for ci, (qc, b) in enumerate(chunk_order):
                acts = [kt for kt in range(NKT) if classes[kt][qc][0] != "n"]
                assert acts, "fully-masked q chunk unsupported"
                q0 = b * S + qc * 512  # global q col base for this chunk
                qs = slice(q0, q0 + 512)
                sigs = {}
                ktc = [0]  # kt-iteration counter across both j loops
                for j in range(2):  # head pairs (2j, 2j+1)
                    pso_e = ps_sm.tile([65, 512], F32, tag="sm", name="pso_e")
                    pso_o = ps_sm.tile([65, 512], F32, tag="sm", name="pso_o")
                    for kt in acts:
                        cl = classes[kt][qc]
                        c0 = cl[1] if cl[0] == "m" else 0
                        ks = slice(b * S + kt * 128, b * S + (kt + 1) * 128)
                        pss = ps_big.tile([128, 2, 512], F32, tag="big")
                        # K=64 row-tiled pair: runs concurrently on PE
                        # tiles T0/T8 (one slot for both heads)
                        nc.tensor.matmul(
                            pss[:, 0, c0:],
                            kTz_lo[0:64, ks],
                            q_sb[0:64, j, q0 + c0 : q0 + 512],
                        )
                        nc.tensor.matmul(
                            pss[:, 1, c0:],
                            kTz_hi[64:128, ks],
                            q_sb[64:128, j, q0 + c0 : q0 + 512],
                        )
                        p_t = small.tile([128, 2, 512], BF16, tag="p_t")
                        nc.scalar.activation(p_t[:, :, c0:], pss[:, :, c0:], EXP)
                        if cl[0] == "m":
                            c1 = cl[2]
                            mkb = all_mks[(kt, qc)][:, None, :].to_broadcast(
                                [128, 2, c1 - c0]
                            )
                            nc.gpsimd.tensor_tensor(
                                p_t[:, :, c0:c1],
                                p_t[:, :, c0:c1],
                                mkb,
                                mybir.AluOpType.mult,
                            )
                        first = kt == acts[0]
                        last = kt == acts[-1]
                        vkt = v_sb[:, b * NKT + kt, :]
                        nc.tensor.matmul(
                            pso_e[:, c0:], vkt, p_t[:, 0, c0:],
                            start=first, stop=last, skip_group_check=True,
                        )
                        nc.tensor.matmul(
                            pso_o[:, c0:], vkt, p_t[:, 1, c0:],
                            start=first, stop=last, skip_group_check=True,
                        )
                        # delay PE-dependent tail items until the previous
                        # chunk's denominator chain has had time to finish
                        if ktc[0] == 0:
                            drain(1)
                        elif ktc[0] >= (2 if len(acts) <= 5 else 3):
                            drain(2)
                        ktc[0] += 1
                    for half, pso in ((0, pso_e), (1, pso_o)):
                        # evacuate the PV bank (frees PSUM early, gives
                        # DMA-able access to the denominator row)
                        sig = small.tile(
                            [65, 512], F32, tag="sig", bufs=6, name=f"sig{half}"
                        )
                        nc.vector.tensor_copy(sig[:], pso[:])
                        di = 32 * (2 * j + half)
                        nc.gpsimd.dma_start(dbuf[di : di + 1, :], sig[64:65, :])
                        sigs[(j, half)] = sig
                # queue this chunk's tail; it is emitted interleaved into
                # the next chunk's kt loop.
                while deferred:
                    deferred.pop(0)()
                pending = []
                sigs_ref = sigs

                def mk_denoms():
                    # batched softmax denominators: rec = exp(-ln(d));
                    # non-d rows of dbuf are 1.0 -> stay finite.
                    nc.scalar.activation(lgb[:], dbuf[:], LN)
                    nc.scalar.activation(recb[:], lgb[:], EXP, scale=-1.0)

                pending.append(mk_denoms)

                def mk_at(j, half, sigs=sigs, qs=qs):
                    sig = sigs[(j, half)]
                    i4 = 2 * j + half
                    pb = ps_sm.tile([64, 512], F32, tag="sm", name="pb")
                    nc.tensor.matmul(pb[:], sel_sb[:, i4, :], recb[:])
                    if half == 0:
                        nc.vector.tensor_mul(
                            at_sb[0:64, j, qs], sig[0:64, :], pb[:]
                        )
                    else:
                        att = small.tile([64, 512], BF16, tag="att")
                        nc.vector.tensor_mul(att[:], sig[0:64, :], pb[:])
                        nc.gpsimd.dma_start(at_sb[64:128, j, qs], att[:])

                from functools import partial

                for j in range(2):
                    for half in range(2):
                        pending.append(partial(mk_at, j, half))

                next_qc = chunk_order[ci + 1][0] if ci + 1 < len(chunk_order) else 3

                def mk_wo(jj, ph, b=b, qc=qc, next_qc=next_qc):
                    mt2 = (b * S + qc * 512) // 128 + jj
                    psd = ps_big.tile([128, 2, 512], F32, tag="big")
                    for sub in range(2):
                        ntc = ph * 2 + sub
                        for ch in range(2):
                            nc.tensor.matmul(
                                psd[:, sub, :],
                                at_sb[:, ch, mt2 * 128 : (mt2 + 1) * 128],
                                wo_sb[:, ch, ntc * 512 : (ntc + 1) * 512],
                                start=(ch == 0),
                                stop=(ch == 1),
                            )
                    ot = small.tile([128, 2, 512], BF16, tag="ot", bufs=4)
                    # these drain into the NEXT chunk: if that chunk is
                    # exp-gated (many z tiles) keep scalar free for exps;
                    # otherwise alternate DVE/ACT to double the evac rate
                    if next_qc >= 2 or (jj + ph) % 2 == 0:
                        nc.vector.tensor_copy(ot[:], psd[:])
                    else:
                        nc.scalar.copy(ot[:], psd[:])
                    nc.sync.dma_start(
                        out_d[
                            mt2 * 128 : (mt2 + 1) * 128,
                            ph * 1024 : (ph + 1) * 1024,
                        ],
                        ot[:].rearrange("p a b -> p (a b)"),
                    )

                for jj in range(4):
                    for ph in range(2):
                        pending.append(partial(mk_wo, jj, ph))
                deferred = pending
            while deferred:
                deferred.pop(0)()
    _split_multi_waits(nc)
    return nc


_NC_CACHE = {}


def _classify_mask(mask):
    """Per (kt, qc) tile class from the [S, S] additive mask ([q, k]).

    Returns classes[kt][qc] in {('z',), ('n',), ('m', c0, c1)} where for 'm'
    tiles q-columns [0,c0) are fully masked, [c0,c1) are mixed (mask tile
    multiplied in), and [c1,512) are fully live.
    """
    classes = []
    for kt in range(NKT):
        row = []
        for qc in range(NQC):
            sub = mask[qc * 512 : (qc + 1) * 512, kt * 128 : (kt + 1) * 128]
            neg_rows = np.all(sub <= NEG_THRESH, axis=1)  # per q col
            zero_rows = np.all(sub == 0.0, axis=1)
            if zero_rows.all():
                row.append(("z",))
            elif neg_rows.all():
                row.append(("n",))
            else:
                # c0: leading run of fully-masked q cols; c1: first index
                # from which all q cols are fully live.
                c0 = 0
                while c0 < 512 and neg_rows[c0]:
                    c0 += 1
                c1 = 512
                while c1 > c0 and zero_rows[c1 - 1]:
                    c1 -= 1
                row.append(("m", int(c0), int(c1)))
        classes.append(row)
    # every q column must keep at least one active k tile covering it,
    # starting from its first active tile (PV 'start' zeroes the bank).
    for qc in range(NQC):
        if all(classes[kt][qc][0] == "n" for kt in range(NKT)):
            for kt in range(NKT):
                classes[kt][qc] = ("m", 0, 512)
        acts = [kt for kt in range(NKT) if classes[kt][qc][0] != "n"]
        first = classes[acts[0]][qc]
        if first[0] == "m" and first[1] != 0:
            classes[acts[0]][qc] = ("m", 0, first[2])
        # verify coverage: col c must be live in some tile with c >= c0
        covered = np.zeros(512, dtype=bool)
        for kt in acts:
            cl = classes[kt][qc]
            covered[(cl[1] if cl[0] == "m" else 0) :] = True
        assert covered.all(), f"uncovered q cols in chunk {qc}"
    return classes


def _prep_inputs(x, freqs_cos, freqs_sin, mask, wq, wk, wv, wo):
    bf = ml_dtypes.bfloat16
    x2 = np.ascontiguousarray(np.asarray(x, dtype=np.float32).reshape(BS, DIM))
    xT = np.ascontiguousarray(x2.T).astype(bf)
    maskT = np.ascontiguousarray(
        np.exp(np.asarray(mask, dtype=np.float32).T)
    ).astype(bf)

    cos = np.asarray(freqs_cos, dtype=np.float32)  # [S, 32]
    sin = np.asarray(freqs_sin, dtype=np.float32)
    # cosd[d, b*S+s] = cos[s, (d%64)//2]; sind alternates -sin/+sin
    d = np.arange(128)
    pair = (d % 64) // 2
    cosd = cos[:, pair].T  # [128, S]
    sgn = np.where(d % 2 == 0, -1.0, 1.0).astype(np.float32)
    sind = sin[:, pair].T * sgn[:, None]
    cosd = np.ascontiguousarray(np.tile(cosd, (1, B))).astype(bf)
    sind = np.ascontiguousarray(np.tile(sind, (1, B))).astype(bf)

    perm = np.zeros((128, 128), dtype=np.float32)
    idx = np.arange(128)
    perm[idx ^ 1, idx] = 1.0
    perm = perm.astype(bf)
    eye64 = np.eye(64, dtype=np.float32).astype(bf)

    wq = np.asarray(wq, dtype=np.float32)
    wk = np.asarray(wk, dtype=np.float32)
    wv = np.asarray(wv, dtype=np.float32)
    wo = np.asarray(wo, dtype=np.float32)

    in_maps = []
    for c in range(N_CORES):
        hs = slice(c * HPC * HEAD_DIM, (c + 1) * HPC * HEAD_DIM)
        ks = slice(c * HEAD_DIM, (c + 1) * HEAD_DIM)
        wqkv = np.concatenate(
            [wq[:, hs] * 0.125, wv[:, ks], wk[:, ks]], axis=1
        )
        in_maps.append(
            {
                "xT": xT,
                "wqkv_c": np.ascontiguousarray(wqkv).astype(bf),
                "wo_c": np.ascontiguousarray(wo[hs, :]).astype(bf),
                "maskT": maskT,
                "cosd": cosd,
                "sind": sind,
                "perm": perm,
                "eye64": eye64,
            }
        )
    return in_maps


def kernel(x, freqs_cos, freqs_sin, mask, wq, wk, wv, wo, _trace=False):
    classes = _classify_mask(np.asarray(mask, dtype=np.float32))
    key = tuple(tuple(r) for r in classes)
    if key not in _NC_CACHE:
        _NC_CACHE[key] = build_nc(classes)
    nc = _NC_CACHE[key]
    in_maps = _prep_inputs(x, freqs_cos, freqs_sin, mask, wq, wk, wv, wo)
    res = run_bass_kernel_spmd(
        nc, in_maps, core_ids=list(range(N_CORES)), trace=_trace
    )
    out = np.zeros((BS, DIM), dtype=np.float32)
    for c in range(N_CORES):
        out += np.asarray(res.results[c]["out_c"], dtype=np.float32)
    out = out.reshape(B, S, DIM)
    if _trace:
        kernel._last_exec_time_ns = res.exec_time_ns
        kernel._last_profile_json = res.profile_json
    return out


# revision 32
# speedup vs baseline: 1.0310x; 1.0310x over previous
"""Self-contained Trainium2 Bass kernel for the GQA attention module.

Sharding: tensor-parallel over heads. Core c owns q-heads [4c..4c+4) and
kv-head c, computes its partial of (attn @ wo); the host sums the 8
partials (the "all-reduce after wo" done host-side during unshard).

Device design (see build_nc):
  - x arrives pre-transposed (xT [DIM, B*S]); Q/K/V projections use one
    packed weight [wq/8 | wv | wk] so K and V share a full 128-row
    M-tile (V^T lands on partitions 0:64 untouched, K^T on 64:128 gets
    RoPE; Q is RoPE'd via cos/sin tables + a pair-swap permutation
    matmul). Projection blocks are emitted just-in-time between
    attention chunks so their DMA/compute overlaps the pipeline.
  - scores are computed transposed (S^T [k, q]) so softmax's P^T is
    directly the moving operand of the PV matmul, and the softmax
    denominator comes free via a ones-column appended to V. The score
    pair for the two heads of a pair runs K=64 row-tiled on PE tiles
    T0/T8 concurrently (kTz_lo/kTz_hi hold K^T in the matching
    partition halves, zero-padded so full-K use also works).
  - mask tiles are classified host-side: all-(-inf) tiles are skipped,
    all-zero tiles skip the mask multiply, and mixed (diagonal) tiles
    are column-trimmed: scores/exp/PV run only on live columns and the
    mask multiply touches only the genuinely mixed sub-block.
  - softmax denominators for all 4 (pair, half) combos of a chunk are
    DMA-gathered to partitions 0/32/64/96 of one tile and Ln/Exp'd in
    two batched ACT ops; the reciprocal rows are broadcast back through
    K=128 selector matmuls.
  - each chunk's tail (denominators, at-normalization, wo matmuls +
    output stores) is deferred and drained into the next chunk's kt
    loop with a ~3-iteration delay, hiding the denominator latency
    chain and filling exp-gated PE slack. PSUM fits in 8 banks via two
    shared ring pools ([128,2,512]x2 and [65,512]x4).
"""
import sys
import types

sys.path.insert(0, "/opt/trn_rl_repo")

import numpy as np
import ml_dtypes


def _install_axon_hook_shim():
    import antenv

    if "antenv.axon_hooks" in sys.modules:
        return
    m = types.ModuleType("antenv.axon_hooks")
    m._hook = None

    def set_axon_ntff_profile_hook(h):
        m._hook = h

    def get_axon_ntff_profile_hook():
        return m._hook

    m.set_axon_ntff_profile_hook = set_axon_ntff_profile_hook
    m.get_axon_ntff_profile_hook = get_axon_ntff_profile_hook
    sys.modules["antenv.axon_hooks"] = m
    antenv.axon_hooks = m
    try:
        from trn_agent_boot.trn_boot import _ntff_profile_via_ctypes

        hook = _ntff_profile_via_ctypes("/opt/axon/libaxon_pjrt.so")
        if hook is not None:
            m.set_axon_ntff_profile_hook(hook)
    except Exception:
        pass


_install_axon_hook_shim()

import concourse.bass as bass
import concourse.mybir as mybir
import concourse.tile as tile
from concourse.bass_utils import run_bass_kernel_spmd

BF16 = mybir.dt.bfloat16
F32 = mybir.dt.float32

B, S, DIM = 2, 2048, 2048
N_HEADS, N_KV_HEADS, HEAD_DIM = 32, 8, 64
N_CORES = 8
HPC = N_HEADS // N_CORES  # 4 q heads per core
BS = B * S  # 4096 rows
NKT = S // 128  # 16 k tiles per batch
NQC = S // 512  # 4 q chunks per batch
NNT = BS // 512  # 8 projection column blocks
NEG_THRESH = -1e4

EXP = mybir.ActivationFunctionType.Exp
LN = mybir.ActivationFunctionType.Ln


def _patched_drain_and_barrier(self, tick_clock, wait_clock):
    # walrus (CoreV3) only accepts one sync-wait on the tile exit drain;
    # split the accumulated waits across single-wait nops.
    nc = self.nc
    drain_inst = nc.sync.drain()
    wait_clock.add_sem_waits(
        drain_inst.ins, tile.ScopedClock({None: tick_clock.global_clock})
    )
    si = drain_inst.ins.sync_info
    sw = list(si.on_wait) if si and si.on_wait else []
    if len(sw) > 1:
        si.on_wait = [sw[0]]
        for w in sw[1:]:
            n2 = nc.sync.nop(nofuse=True)
            if n2.ins.sync_info is None:
                n2.ins.sync_info = mybir.SyncInfo(on_wait=[w], on_update=[])
            else:
                n2.ins.sync_info.on_wait = [w]
    nc.all_engine_barrier()
    assert self.sems is not None
    popped = nc._tile_sem_poison_stack.pop()
    assert popped is self._sem_poison
    nc.clear_and_free_semaphores(list(self.sems.allocated().values()))
    nc.all_engine_barrier()


tile.TileContext._drain_and_barrier = _patched_drain_and_barrier


def _split_multi_waits(nc):
    """walrus (this build) accepts at most one sync-wait per instruction;
    move extra waits onto same-engine nops inserted just before."""
    n_split = 0
    for f in nc.m.functions:
        for blk in f.blocks:
            new_insts = []
            for inst in blk.instructions:
                si = getattr(inst, "sync_info", None)
                if si is not None and si.on_wait and len(si.on_wait) > 1:
                    extra = list(si.on_wait[:-1])
                    si.on_wait = [si.on_wait[-1]]
                    for w in extra:
                        nop = mybir.InstNoOp(
                            name=nc.get_next_instruction_name(), ins=[], outs=[]
                        )
                        nop.engine = inst.engine
                        nop.sync_info = mybir.SyncInfo(on_wait=[w], on_update=[])
                        new_insts.append(nop)
                        n_split += 1
                new_insts.append(inst)
            blk.instructions[:] = new_insts
    return n_split


def build_nc(classes, debug_phase=None):
    """classes[kt][qc]: ('z',) all-live / ('n',) dead / ('m', c0, c1) with
    q-columns [0,c0) dead, [c0,c1) mixed (mask applied), [c1,512) live."""
    nc = bass.Bass("TRN2", target_bir_lowering=False, debug=False, num_devices=N_CORES)

    xT_d = nc.dram_tensor("xT", [DIM, BS], BF16, kind="ExternalInput")
    # packed [wq*(1/8) | wv | wk]: V at output partitions 0:64, K at 64:128
    wqkv_d = nc.dram_tensor("wqkv_c", [DIM, 384], BF16, kind="ExternalInput")
    wo_d = nc.dram_tensor("wo_c", [HPC * HEAD_DIM, DIM], BF16, kind="ExternalInput")
    maskT_d = nc.dram_tensor("maskT", [S, S], BF16, kind="ExternalInput")
    cosd_d = nc.dram_tensor("cosd", [128, BS], BF16, kind="ExternalInput")
    sind_d = nc.dram_tensor("sind", [128, BS], BF16, kind="ExternalInput")
    perm_d = nc.dram_tensor("perm", [128, 128], BF16, kind="ExternalInput")
    eye64_d = nc.dram_tensor("eye64", [64, 64], BF16, kind="ExternalInput")
    out_d = nc.dram_tensor("out_c", [BS, DIM], BF16, kind="ExternalOutput")

    # chunk schedule: C_i = (qc, b); chunk (qc, b) needs proj block
    # nt = b*4 + qc (its own Q rows) and all earlier nts of batch b (K/V).
    chunk_order = [(qc, b) for qc in range(NQC) for b in range(B)]
    chunk_order.sort(key=lambda cb: (cb[0], cb[1]))
    # processing order (0,0),(0,1),(1,0),... -> nt need order 0,4,1,5,2,6,3,7
    nt_for_chunk = [b * NQC + qc for (qc, b) in chunk_order]

    with tile.TileContext(nc) as tc:
        with (
            tc.tile_pool(name="persist", bufs=1) as persist,
            tc.tile_pool(name="stream", bufs=3) as stream,
            tc.tile_pool(name="small", bufs=5) as small,
            tc.tile_pool(name="maskpool", bufs=16) as mp,
            tc.tile_pool(name="ps_big", bufs=2, space="PSUM") as ps_big,
            tc.tile_pool(name="ps_sm", bufs=4, space="PSUM") as ps_sm,
        ):
            # ---- persistent tensors ----
            wqkv_sb = persist.tile([128, NKT, 384], BF16, tag="wqkv")
            wo_sb = persist.tile([128, 2, DIM], BF16, tag="wo")
            perm_sb = persist.tile([128, 128], BF16, tag="perm")
            eye64_sb = persist.tile([64, 64], BF16, tag="eye64")
            q_sb = persist.tile([128, 2, NNT * 512], BF16, tag="q")  # Q^T
            # K^T zero-padded to K=128 so scores run in full (non-tiled)
            # PE mode: kTz_lo = [K^T; 0] for even heads, kTz_hi = [0; K^T].
            kTz_lo = persist.tile([128, BS], BF16, tag="kTlo")
            kTz_hi = persist.tile([128, BS], BF16, tag="kThi")
            v_sb = persist.tile([128, B * NKT, 65], BF16, tag="v")  # [V|1]
            at_sb = persist.tile([128, 2, BS], BF16, tag="at")  # A^T
            # selector rows for the K=128 denominator-broadcast matmuls
            sel_sb = persist.tile([128, 4, 64], BF16, tag="sel")
            dbuf = persist.tile([128, 512], F32, tag="dbuf")  # d rows at 0/32/64/96
            lgb = persist.tile([128, 512], F32, tag="lgb")
            recb = persist.tile([128, 512], BF16, tag="recb")

            for tch in range(4):
                nc.gpsimd.dma_start(
                    wqkv_sb[:, 4 * tch : 4 * tch + 4, :],
                    wqkv_d[4 * tch * 128 : (4 * tch + 4) * 128, :].rearrange(
                        "(t p) m -> p t m", p=128
                    ),
                )
            nc.gpsimd.dma_start(perm_sb[:], perm_d[:])
            nc.gpsimd.dma_start(eye64_sb[:], eye64_d[:])
            nc.gpsimd.dma_start(wo_sb[:], wo_d.rearrange("(t p) m -> p t m", p=128))
            nc.gpsimd.memset(v_sb[:, :, 64:65], 1.0)
            nc.gpsimd.memset(kTz_lo[64:128, :], 0.0)
            nc.gpsimd.memset(kTz_hi[0:64, :], 0.0)
            nc.gpsimd.memset(sel_sb[:], 0.0)
            for i in range(4):
                nc.gpsimd.memset(sel_sb[32 * i : 32 * i + 1, i, :], 1.0)
            nc.gpsimd.memset(dbuf[:], 1.0)
            # prefetch all mixed-mask tiles
            all_mks = {}
            for qc in range(NQC):
                for kt in range(NKT):
                    cl = classes[kt][qc]
                    if cl[0] == "m":
                        c0, c1 = cl[1], cl[2]
                        mk = mp.tile(
                            [128, c1 - c0], BF16, tag=f"mk{c1 - c0}", name="mk"
                        )
                        nc.gpsimd.dma_start(
                            mk[:],
                            maskT_d[
                                kt * 128 : (kt + 1) * 128,
                                qc * 512 + c0 : qc * 512 + c1,
                            ],
                        )
                        all_mks[(kt, qc)] = mk

            # ---- projection block emission (interleaved between chunks) ----
            xblks = {}

            def load_nt(nt, spread=False):
                cs = slice(nt * 512, (nt + 1) * 512)
                xblk = stream.tile([128, NKT, 512], BF16, tag="xblk")
                for tch in range(4):
                    eng = nc.scalar if (spread and tch % 2) else nc.sync
                    eng.dma_start(
                        xblk[:, 4 * tch : 4 * tch + 4, :],
                        xT_d[4 * tch * 128 : (4 * tch + 4) * 128, cs].rearrange(
                            "(t p) n -> p t n", p=128
                        ),
                    )
                cosb = stream.tile([128, 512], BF16, tag="cosb")
                sinb = stream.tile([128, 512], BF16, tag="sinb")
                nc.sync.dma_start(cosb[:], cosd_d[:, cs])
                nc.sync.dma_start(sinb[:], sind_d[:, cs])
                xblks[nt] = (xblk, cosb, sinb)

            def emit_proj(nt):
                cs = slice(nt * 512, (nt + 1) * 512)
                xblk, cosb, sinb = xblks.pop(nt)
                # Q: 2 M-tiles (the 1/8 scale is folded into wq host-side)
                for mt in range(2):
                    psqb = ps_big.tile([128, 2, 512], F32, tag="big", name="psq")
                    psq = psqb[:, 0, :]
                    for kt in range(NKT):
                        nc.tensor.matmul(
                            psq,
                            wqkv_sb[:, kt, mt * 128 : (mt + 1) * 128],
                            xblk[:, kt, :],
                            start=(kt == 0),
                            stop=(kt == NKT - 1),
                        )
                    q_tmp = small.tile([128, 512], BF16, tag="q_tmp")
                    nc.scalar.copy(q_tmp[:], psq)
                    psw = ps_sm.tile([128, 512], F32, tag="sm", name="psw")
                    nc.tensor.matmul(psw[:], perm_sb[:], q_tmp[:])
                    v1 = small.tile([128, 512], BF16, tag="v1")
                    nc.vector.tensor_mul(v1[:], q_tmp[:], cosb[:])
                    v2 = small.tile([128, 512], BF16, tag="v2")
                    nc.vector.tensor_mul(v2[:], psw[:], sinb[:])
                    nc.vector.tensor_add(q_sb[:, mt, cs], v1[:], v2[:])
                # fused [V | K] projection: V^T on partitions 0:64 (no RoPE,
                # transposed as-is), K^T on 64:128 (RoPE'd)
                pskb = ps_big.tile([128, 2, 512], F32, tag="big", name="psk")
                psk = pskb[:, 0, :]
                for kt in range(NKT):
                    nc.tensor.matmul(
                        psk,
                        wqkv_sb[:, kt, 256:384],
                        xblk[:, kt, :],
                        start=(kt == 0),
                        stop=(kt == NKT - 1),
                    )
                k_tmp = small.tile([128, 512], BF16, tag="k_tmp")
                nc.scalar.copy(k_tmp[:], psk)
                pskw = ps_sm.tile([128, 512], F32, tag="sm", name="pskw")
                nc.tensor.matmul(
                    pskw[64:128, :],
                    perm_sb[64:128, 64:128],
                    k_tmp[64:128, :],
                )
                kv1 = small.tile([128, 512], BF16, tag="kv1")
                nc.vector.tensor_mul(
                    kv1[64:128, :], k_tmp[64:128, :], cosb[64:128, :]
                )
                kv2 = small.tile([128, 512], BF16, tag="kv2")
                nc.vector.tensor_mul(
                    kv2[64:128, :], pskw[64:128, :], sinb[64:128, :]
                )
                nc.vector.tensor_add(
                    kTz_hi[64:128, cs], kv1[64:128, :], kv2[64:128, :]
                )
                nc.gpsimd.dma_start(kTz_lo[0:64, cs], kTz_hi[64:128, cs])
                for jt in range(4):
                    pst = ps_sm.tile([128, 512], BF16, tag="sm", name="pst")
                    nc.tensor.transpose(
                        pst[:, 0:64], k_tmp[0:64, jt * 128 : (jt + 1) * 128],
                        eye64_sb[:],
                    )
                    rc = nt * 4 + jt
                    nc.scalar.copy(v_sb[:, rc, 0:64], pst[:, 0:64])

            # ---- main pipeline over chunks ----
            deferred = []

            def drain(n):
                for _ in range(n):
                    if deferred:
                        deferred.pop(0)()

            load_nt(nt_for_chunk[0], spread=True)
            load_nt(nt_for_chunk[1], spread=True)
            for ci in range(len(chunk_order)):
                if ci + 2 < len(chunk_order):
                    load_nt(nt_for_chunk[ci + 2])
                emit_proj(nt_for_chunk[ci])

            for ci, (qc, b) in enumerate(chunk_order):
                acts = [kt for kt in range(NKT) if classes[kt][qc][0] != "n"]
                assert acts, "fully-masked q chunk unsupported"
                q0 = b * S + qc * 512  # global q col base for this chunk
                qs = slice(q0, q0 + 512)
                sigs = {}
                ktc = [0]  # kt-iteration counter across both j loops
                for j in range(2):  # head pairs (2j, 2j+1)
                    pso_e = ps_sm.tile([65, 512], F32, tag="sm", name="pso_e")
                    pso_o = ps_sm.tile([65, 512], F32, tag="sm", name="pso_o")
                    for kt in acts:
                        cl = classes[kt][qc]
                        c0 = cl[1] if cl[0] == "m" else 0
                        ks = slice(b * S + kt * 128, b * S + (kt + 1) * 128)
                        pss = ps_big.tile([128, 2, 512], F32, tag="big")
                        # K=64 row-tiled pair: runs concurrently on PE
                        # tiles T0/T8 (one slot for both heads)
                        nc.tensor.matmul(
                            pss[:, 0, c0:],
                            kTz_lo[0:64, ks],
                            q_sb[0:64, j, q0 + c0 : q0 + 512],
                        )
                        nc.tensor.matmul(
                            pss[:, 1, c0:],
                            kTz_hi[64:128, ks],
                            q_sb[64:128, j, q0 + c0 : q0 + 512],
                        )
                        p_t = small.tile([128, 2, 512], BF16, tag="p_t")
                        nc.scalar.activation(p_t[:, :, c0:], pss[:, :, c0:], EXP)
                        if cl[0] == "m":
                            c1 = cl[2]
                            mkb = all_mks[(kt, qc)][:, None, :].to_broadcast(
                                [128, 2, c1 - c0]
                            )
                            nc.vector.tensor_tensor(
                                p_t[:, :, c0:c1],
                                p_t[:, :, c0:c1],
                                mkb,
                                mybir.AluOpType.mult,
                            )
                        first = kt == acts[0]
                        last = kt == acts[-1]
                        vkt = v_sb[:, b * NKT + kt, :]
                        nc.tensor.matmul(
                            pso_e[:, c0:], vkt, p_t[:, 0, c0:],
                            start=first, stop=last, skip_group_check=True,
                        )
                        nc.tensor.matmul(
                            pso_o[:, c0:], vkt, p_t[:, 1, c0:],
                            start=first, stop=last, skip_group_check=True,
                        )
                        # delay PE-dependent tail items until the previous
                        # chunk's denominator chain has had time to finish
                        if ktc[0] == 0:
                            drain(1)
                        elif ktc[0] >= (2 if len(acts) <= 5 else 3):
                            drain(2)
                        ktc[0] += 1
                    for half, pso in ((0, pso_e), (1, pso_o)):
                        # evacuate the PV bank (frees PSUM early, gives
                        # DMA-able access to the denominator row)
                        sig = small.tile(
                            [65, 512], F32, tag="sig", bufs=6, name=f"sig{half}"
                        )
                        nc.vector.tensor_copy(sig[:], pso[:])
                        di = 32 * (2 * j + half)
                        nc.gpsimd.dma_start(dbuf[di : di + 1, :], sig[64:65, :])
                        sigs[(j, half)] = sig
                # queue this chunk's tail; it is emitted interleaved into
                # the next chunk's kt loop.
                while deferred:
                    deferred.pop(0)()
                pending = []
                sigs_ref = sigs

                def mk_denoms():
                    # batched softmax denominators: rec = exp(-ln(d));
                    # non-d rows of dbuf are 1.0 -> stay finite.
                    nc.scalar.activation(lgb[:], dbuf[:], LN)
                    nc.scalar.activation(recb[:], lgb[:], EXP, scale=-1.0)

                pending.append(mk_denoms)

                def mk_at(j, half, sigs=sigs, qs=qs):
                    sig = sigs[(j, half)]
                    i4 = 2 * j + half
                    pb = ps_sm.tile([64, 512], F32, tag="sm", name="pb")
                    nc.tensor.matmul(pb[:], sel_sb[:, i4, :], recb[:])
                    if half == 0:
                        nc.vector.tensor_mul(
                            at_sb[0:64, j, qs], sig[0:64, :], pb[:]
                        )
                    else:
                        att = small.tile([64, 512], BF16, tag="att")
                        nc.vector.tensor_mul(att[:], sig[0:64, :], pb[:])
                        nc.gpsimd.dma_start(at_sb[64:128, j, qs], att[:])

                from functools import partial

                for j in range(2):
                    for half in range(2):
                        pending.append(partial(mk_at, j, half))

                next_qc = chunk_order[ci + 1][0] if ci + 1 < len(chunk_order) else 3

                def mk_wo(jj, ph, b=b, qc=qc, next_qc=next_qc):
                    mt2 = (b * S + qc * 512) // 128 + jj
                    psd = ps_big.tile([128, 2, 512], F32, tag="big")
                    for sub in range(2):
                        ntc = ph * 2 + sub
                        for ch in range(2):
                            nc.tensor.matmul(
                                psd[:, sub, :],
                                at_sb[:, ch, mt2 * 128 : (mt2 + 1) * 128],
                                wo_sb[:, ch, ntc * 512 : (ntc + 1) * 512],
                                start=(ch == 0),
                                stop=(ch == 1),
                            )
                    ot = small.tile([128, 2, 512], BF16, tag="ot", bufs=4)
                    # these drain into the NEXT chunk: if that chunk is
                    # exp-gated (many z tiles) keep scalar free for exps;
                    # otherwise alternate DVE/ACT to double the evac rate
                    if next_qc >= 2 or (jj + ph) % 2 == 0:
                        nc.vector.tensor_copy(ot[:], psd[:])
                    else:
                        nc.scalar.copy(ot[:], psd[:])
                    nc.sync.dma_start(
                        out_d[
                            mt2 * 128 : (mt2 + 1) * 128,
                            ph * 1024 : (ph + 1) * 1024,
                        ],
                        ot[:].rearrange("p a b -> p (a b)"),
                    )

                for jj in range(4):
                    for ph in range(2):
                        pending.append(partial(mk_wo, jj, ph))
                deferred = pending
            while deferred:
                deferred.pop(0)()
    _split_multi_waits(nc)
    return nc


_NC_CACHE = {}


def _classify_mask(mask):
    """Per (kt, qc) tile class from the [S, S] additive mask ([q, k]).

    Returns classes[kt][qc] in {('z',), ('n',), ('m', c0, c1)} where for 'm'
    tiles q-columns [0,c0) are fully masked, [c0,c1) are mixed (mask tile
    multiplied in), and [c1,512) are fully live.
    """
    classes = []
    for kt in range(NKT):
        row = []
        for qc in range(NQC):
            sub = mask[qc * 512 : (qc + 1) * 512, kt * 128 : (kt + 1) * 128]
            neg_rows = np.all(sub <= NEG_THRESH, axis=1)  # per q col
            zero_rows = np.all(sub == 0.0, axis=1)
            if zero_rows.all():
                row.append(("z",))
            elif neg_rows.all():
                row.append(("n",))
            else:
                # c0: leading run of fully-masked q cols; c1: first index
                # from which all q cols are fully live.
                c0 = 0
                while c0 < 512 and neg_rows[c0]:
                    c0 += 1
                c1 = 512
                while c1 > c0 and zero_rows[c1 - 1]:
                    c1 -= 1
                row.append(("m", int(c0), int(c1)))
        classes.append(row)
    # every q column must keep at least one active k tile covering it,
    # starting from its first active tile (PV 'start' zeroes the bank).
    for qc in range(NQC):
        if all(classes[kt][qc][0] == "n" for kt in range(NKT)):
            for kt in range(NKT):
                classes[kt][qc] = ("m", 0, 512)
        acts = [kt for kt in range(NKT) if classes[kt][qc][0] != "n"]
        first = classes[acts[0]][qc]
        if first[0] == "m" and first[1] != 0:
            classes[acts[0]][qc] = ("m", 0, first[2])
        # verify coverage: col c must be live in some tile with c >= c0
        covered = np.zeros(512, dtype=bool)
        for kt in acts:
            cl = classes[kt][qc]
            covered[(cl[1] if cl[0] == "m" else 0) :] = True
        assert covered.all(), f"uncovered q cols in chunk {qc}"
    return classes


def _prep_inputs(x, freqs_cos, freqs_sin, mask, wq, wk, wv, wo):
    bf = ml_dtypes.bfloat16
    x2 = np.ascontiguousarray(np.asarray(x, dtype=np.float32).reshape(BS, DIM))
    xT = np.ascontiguousarray(x2.T).astype(bf)
    maskT = np.ascontiguousarray(
        np.exp(np.asarray(mask, dtype=np.float32).T)
    ).astype(bf)

    cos = np.asarray(freqs_cos, dtype=np.float32)  # [S, 32]
    sin = np.asarray(freqs_sin, dtype=np.float32)
    # cosd[d, b*S+s] = cos[s, (d%64)//2]; sind alternates -sin/+sin
    d = np.arange(128)
    pair = (d % 64) // 2
    cosd = cos[:, pair].T  # [128, S]
    sgn = np.where(d % 2 == 0, -1.0, 1.0).astype(np.float32)
    sind = sin[:, pair].T * sgn[:, None]
    cosd = np.ascontiguousarray(np.tile(cosd, (1, B))).astype(bf)
    sind = np.ascontiguousarray(np.tile(sind, (1, B))).astype(bf)

    perm = np.zeros((128, 128), dtype=np.float32)
    idx = np.arange(128)
    perm[idx ^ 1, idx] = 1.0
    perm = perm.astype(bf)
    eye64 = np.eye(64, dtype=np.float32).astype(bf)

    wq = np.asarray(wq, dtype=np.float32)
    wk = np.asarray(wk, dtype=np.float32)
    wv = np.asarray(wv, dtype=np.float32)
    wo = np.asarray(wo, dtype=np.float32)

    in_maps = []
    for c in range(N_CORES):
        hs = slice(c * HPC * HEAD_DIM, (c + 1) * HPC * HEAD_DIM)
        ks = slice(c * HEAD_DIM, (c + 1) * HEAD_DIM)
        wqkv = np.concatenate(
            [wq[:, hs] * 0.125, wv[:, ks], wk[:, ks]], axis=1
        )
        in_maps.append(
            {
                "xT": xT,
                "wqkv_c": np.ascontiguousarray(wqkv).astype(bf),
                "wo_c": np.ascontiguousarray(wo[hs, :]).astype(bf),
                "maskT": maskT,
                "cosd": cosd,
                "sind": sind,
                "perm": perm,
                "eye64": eye64,
            }
        )
    return in_maps


def kernel(x, freqs_cos, freqs_sin, mask, wq, wk, wv, wo, _trace=False):
    classes = _classify_mask(np.asarray(mask, dtype=np.float32))
    key = tuple(tuple(r) for r in classes)
    if key not in _NC_CACHE:
        _NC_CACHE[key] = build_nc(classes)
    nc = _NC_CACHE[key]
    in_maps = _prep_inputs(x, freqs_cos, freqs_sin, mask, wq, wk, wv, wo)
    res = run_bass_kernel_spmd(
        nc, in_maps, core_ids=list(range(N_CORES)), trace=_trace
    )
    out = np.zeros((BS, DIM), dtype=np.float32)
    for c in range(N_CORES):
        out += np.asarray(res.results[c]["out_c"], dtype=np.float32)
    out = out.reshape(B, S, DIM)
    if _trace:
        kernel._last_exec_time_ns = res.exec_time_ns
        kernel._last_profile_json = res.profile_json
    return out
